# revision 1
# baseline (speedup 1.0000x reference)
"""Trainium2 Bass kernel for the CJEPA recurrent slot model.

Full-input contract: kernel(**inputs) takes the complete (unsharded) numpy
arrays and returns the full (B, T, N, D) output. Internally the batch is
sharded 4-per-core across 8 NeuronCores; the small parameter set is
replicated.

Per-core plan (v2):
  Phase 1 (parallel over time, 32-step chunks of 128 rows=(b,t)):
  z = tanh(obs @ W_enc.T + b_enc) via PE with xbar-transposed obs;
  K/V; all 16 slot queries as 8 N=512 matmuls + broadcast-bias add;
  sigmoid attention logits via DVE accumulate; slot blend on Pool engine;
  L2 normalize via one ACT Rsqrt; alpha-prescaled A_t = a*normalize(shat)
  written bf16 to DRAM scratch in natural [t][i=(b,n)][d] layout.
  Phase 2 (sequential over T): transposed-domain recurrence
  S_t = A_t + (1-a)*tanh(W_temporal S_{t-1}); per step only the four
  128x128x64 bf16 matmuls run on PE; tanh on ACT; blend on DVE. All
  layout changes ride the DMA xbar: scratch->spT (2 calls/chunk) and
  S-strip -> natural output strip (1 call/8 steps) + batched bf16 output
  DMAs (4/8 steps).
"""

from contextlib import ExitStack

import numpy as np

B, T_FULL, D_OBS, D, NV = 32, 256, 1024, 256, 16
N_CORES = 8
B_LOC = B // N_CORES        # 4
I_DIM = B_LOC * NV          # 64 recurrent sequences per core
ALPHA = 0.7

_CACHE = {}


def build(T=T_FULL):
    import concourse.tile as tile
    from concourse import bacc, masks, mybir

    F32 = mybir.dt.float32
    BF = mybir.dt.bfloat16
    AF = mybir.ActivationFunctionType
    OP = mybir.AluOpType

    n_chunks = T // 32

    nc = bacc.Bacc("TRN2", target_bir_lowering=False, debug=False,
                   num_devices=N_CORES)
    obs_v = nc.dram_tensor("observations", [B_LOC, T, D_OBS], F32,
                           kind="ExternalInput").ap()
    wenc_v = nc.dram_tensor("W_enc", [D, D_OBS], F32,
                            kind="ExternalInput").ap()
    benc_v = nc.dram_tensor("b_enc", [D, 1], F32, kind="ExternalInput").ap()
    wkey_v = nc.dram_tensor("W_key", [D, D], F32, kind="ExternalInput").ap()
    wval_v = nc.dram_tensor("W_value", [D, D], F32,
                            kind="ExternalInput").ap()
    wqry_v = nc.dram_tensor("W_query", [NV, D, D], F32,
                            kind="ExternalInput").ap()
    bqry_v = nc.dram_tensor("b_query", [1, NV * D], F32,
                            kind="ExternalInput").ap()
    wtmp_v = nc.dram_tensor("W_temporal", [D, D], F32,
                            kind="ExternalInput").ap()
    out_v = nc.dram_tensor("out", [B_LOC, T, NV, D], BF,
                           kind="ExternalOutput").ap()

    with tile.TileContext(nc) as tc, ExitStack() as ctx:
        const = ctx.enter_context(tc.tile_pool(name="const", bufs=1))
        wpool = ctx.enter_context(tc.tile_pool(name="wpool", bufs=1))
        wtmp_pool = ctx.enter_context(tc.tile_pool(name="wtmp", bufs=2))
        p1 = ctx.enter_context(tc.tile_pool(name="p1", bufs=2))
        small = ctx.enter_context(tc.tile_pool(name="small", bufs=8))
        p2 = ctx.enter_context(tc.tile_pool(name="p2", bufs=3))
        spx = ctx.enter_context(tc.tile_pool(name="spx", bufs=3))
        dramp = ctx.enter_context(tc.tile_pool(name="dramp", bufs=1,
                                               space="DRAM"))
        ps1 = ctx.enter_context(tc.tile_pool(name="ps1", bufs=2,
                                             space="PSUM"))
        ps2 = ctx.enter_context(tc.tile_pool(name="ps2", bufs=1,
                                             space="PSUM"))
        ps3 = ctx.enter_context(tc.tile_pool(name="ps3", bufs=1,
                                             space="PSUM"))
        psq = ctx.enter_context(tc.tile_pool(name="psq", bufs=1,
                                             space="PSUM"))

        scratch = dramp.tile([T, I_DIM, D], BF, tag="scratch")

        ident = const.tile([128, 128], BF, tag="ident")
        masks.make_identity(nc, ident[:])
        ones1 = const.tile([1, 128], BF, tag="ones1")
        nc.vector.memset(ones1[:], 1.0)

        benc = []
        for h in range(2):
            t_ = const.tile([128, 1], F32, tag=f"benc{h}")
            nc.sync.dma_start(t_[:], benc_v[h * 128:(h + 1) * 128, :])
            benc.append(t_)

        def copy_ps(dst, src, use_act):
            if use_act:
                nc.scalar.copy(dst, src)
            else:
                nc.vector.tensor_copy(dst, src)

        def prep_wT_into(dram_ap, rows, cols, strip, col_of, name):
            """dram (rows=k, cols=d) f32 -> bf16 W.T chunks written into
            `strip` at columns col_of(j, rc) (128 wide each)."""
            cj = cols // 128
            rj = rows // 128
            for rc in range(rj):
                nat = wtmp_pool.tile([128, cols], F32, tag="w_nat")
                nc.sync.dma_start(nat[:], dram_ap[rc * 128:(rc + 1) * 128, :])
                natb = wtmp_pool.tile([128, cols], BF, tag="w_natb")
                nc.vector.tensor_copy(natb[:], nat[:])
                for j in range(cj):
                    ps = ps1.tile([128, 128], BF, tag="t1")
                    nc.tensor.transpose(ps[:], natb[:, j * 128:(j + 1) * 128],
                                        ident[:])
                    c0 = col_of(j, rc)
                    copy_ps(strip[:, c0:c0 + 128], ps[:],
                            use_act=((j + rc) % 2 == 0))

        # encoder weights: 8 tiles of (128 dobs, 256 dlat)
        wencT = wpool.tile([128, 8 * D], BF, tag="wencT")
        prep_wT_into(wenc_v, D, D_OBS, wencT,
                     lambda j, rc: j * D + rc * 128, "enc")
        # key/value/temporal: 2 tiles of (128 dlat_in, 256 dlat_out)
        wkeyT = wpool.tile([128, 2 * D], BF, tag="wkeyT")
        prep_wT_into(wkey_v, D, D, wkeyT,
                     lambda j, rc: j * D + rc * 128, "key")
        wvalT = wpool.tile([128, 2 * D], BF, tag="wvalT")
        prep_wT_into(wval_v, D, D, wvalT,
                     lambda j, rc: j * D + rc * 128, "val")
        wtT = wpool.tile([128, 2 * D], BF, tag="wtT")
        prep_wT_into(wtmp_v, D, D, wtT,
                     lambda j, rc: j * D + rc * 128, "tmp")
        # query weights: strip col = j*4096 + n*256 + rc*128 so that the
        # rhs for slot-pair p, contraction chunk j is the contiguous 512
        # columns at j*4096 + p*512.
        wqT = wpool.tile([128, 2 * NV * D], BF, tag="wqT")
        for n in range(NV):
            prep_wT_into(wqry_v[n], D, D, wqT,
                         lambda j, rc, n=n: j * NV * D + n * D + rc * 128,
                         f"q{n}")

        # broadcast bias: bias_bcast[p, n*256+d] = b_query[n, d]
        bq_f = const.tile([1, NV * D], F32, tag="bq_f")
        nc.sync.dma_start(bq_f[:], bqry_v[:])
        bq_bf = const.tile([1, NV * D], BF, tag="bq_bf")
        nc.vector.tensor_copy(bq_bf[:], bq_f[:])
        bias_bcast = const.tile([128, NV * D], BF, tag="bias_bcast")
        for g in range(8):
            ps = psq.tile([128, 512], F32, tag="qps")
            nc.tensor.matmul(ps[:], lhsT=ones1[:],
                             rhs=bq_bf[0:1, g * 512:(g + 1) * 512],
                             start=True, stop=True)
            nc.vector.tensor_copy(bias_bcast[:, g * 512:(g + 1) * 512], ps[:])

        scratch_flat = scratch[:].rearrange("t i d -> t (i d)")

        def newton_rsqrt07(ss):
            """(128,16) f32 sum-of-squares -> ALPHA/max(sqrt(ss),1e-8)."""
            I32 = mybir.dt.int32
            ssc = small.tile([128, NV], F32, tag="nw")
            nc.vector.tensor_scalar(ssc[:], ss[:], 1e-16, None, op0=OP.max)
            sh = small.tile([128, NV], I32, tag="nwi")
            nc.vector.tensor_scalar(sh[:], ssc[:].bitcast(I32), 1, None,
                                    op0=OP.logical_shift_right)
            yi = small.tile([128, NV], I32, tag="nwi")
            nc.vector.tensor_scalar(yi[:], sh[:], -1, 0x5F3759DF,
                                    op0=OP.mult, op1=OP.add)
            y = yi[:].bitcast(F32)
            rn = None
            for it in range(3):
                t1 = small.tile([128, NV], F32, tag="nw")
                nc.vector.tensor_tensor(t1[:], y, y, op=OP.mult)
                t2 = small.tile([128, NV], F32, tag="nw")
                nc.vector.scalar_tensor_tensor(t2[:], in0=t1[:], scalar=-0.5,
                                               in1=ssc[:], op0=OP.mult,
                                               op1=OP.mult)
                t3 = small.tile([128, NV], F32, tag="nw")
                nc.vector.tensor_scalar(t3[:], t2[:], 1.5, None, op0=OP.add)
                if it < 2:
                    yn = small.tile([128, NV], F32, tag="nw")
                    nc.vector.tensor_tensor(yn[:], y, t3[:], op=OP.mult)
                    y = yn[:]
                else:
                    rn = small.tile([128, NV], F32, tag="rn")
                    nc.vector.scalar_tensor_tensor(rn[:], in0=t3[:],
                                                   scalar=ALPHA, in1=y,
                                                   op0=OP.mult, op1=OP.mult)
            return rn

        def phase1_load(c):
            obs_nat = p1.tile([128, D_OBS], F32, tag="obs_nat")
            for b in range(B_LOC):
                nc.gpsimd.dma_start(obs_nat[b * 32:(b + 1) * 32, :],
                                    obs_v[b, c * 32:(c + 1) * 32, :])
            return obs_nat

        def phase1_compute(c, obs_nat):
            """Generator: yields after small instruction quanta so the main
            loop can interleave issue with phase-2 steps (in-order engine
            queues => head-of-line blocking if issued in one burst)."""
            obs_bf = p1.tile([128, D_OBS], BF, tag="obs_bf")
            nc.vector.tensor_copy(obs_bf[:], obs_nat[:])
            # obs transpose via xbar: obsT[do, j*128+r] = obs_bf[r, j*128+do]
            obsT = p1.tile([128, 8 * 128], BF, tag="obsT")
            nc.sync.dma_start_transpose(
                obsT[:].rearrange("p (j r) -> p j r", r=128), obs_bf[:])
            yield

            zT = []
            for h in range(2):
                zp = ps2.tile([128, 128], F32, tag="t2")
                for j in range(8):
                    nc.tensor.matmul(zp[:],
                                     lhsT=wencT[:, j * D + h * 128:
                                                j * D + (h + 1) * 128],
                                     rhs=obsT[:, j * 128:(j + 1) * 128],
                                     start=(j == 0), stop=(j == 7))
                    if j == 3:
                        yield
                zt = p1.tile([128, 128], BF, tag=f"zT{h}")
                nc.scalar.activation(zt[:], zp[:], AF.Tanh,
                                     bias=benc[h][:, 0:1])
                zT.append(zt)
                yield

            kv = {}
            for nm, wT in (("K", wkeyT), ("V", wvalT)):
                ps = ps3.tile([128, D], F32, tag="t3")
                for h in range(2):
                    nc.tensor.matmul(ps[:], lhsT=zT[h][:],
                                     rhs=wT[:, h * D:(h + 1) * D],
                                     start=(h == 0), stop=(h == 1))
                t_ = p1.tile([128, D], BF, tag=f"{nm}_bf", name=f"{nm}_bf")
                nc.vector.tensor_copy(t_[:], ps[:])
                kv[nm] = t_
                yield

            # all 16 slot queries: 8 pair-matmuls of N=512 + bias add
            q_all = p1.tile([128, NV * D], BF, tag="q_all")
            for p in range(8):
                qp = psq.tile([128, 512], F32, tag="qps")
                for j in range(2):
                    nc.tensor.matmul(qp[:],
                                     lhsT=zT[j][:],
                                     rhs=wqT[:, j * NV * D + p * 512:
                                             j * NV * D + (p + 1) * 512],
                                     start=(j == 0), stop=(j == 1))
                nc.vector.tensor_tensor(q_all[:, p * 512:(p + 1) * 512],
                                        qp[:],
                                        bias_bcast[:, p * 512:(p + 1) * 512],
                                        op=OP.add)
                yield

            logits = small.tile([128, NV], F32, tag="logits")
            junk = p1.tile([128, D], BF, tag="junk")
            for n in range(NV):
                nc.vector.scalar_tensor_tensor(
                    junk[:], in0=q_all[:, n * D:(n + 1) * D],
                    scalar=1.0 / 16.0, in1=kv["K"][:],
                    op0=OP.mult, op1=OP.mult,
                    accum_out=logits[:, n:n + 1])
                if n % 2 == 1:
                    yield

            attn = small.tile([128, NV], F32, tag="attn")
            nc.scalar.activation(attn[:], logits[:], AF.Sigmoid)
            yield

            shat = p1.tile([128, NV * D], BF, tag="shat")
            vmq = p1.tile([128, D], BF, tag="vmq")
            ss = small.tile([128, NV], F32, tag="ss")
            junk2 = p1.tile([128, D], BF, tag="junk2")
            for n in range(NV):
                qs = q_all[:, n * D:(n + 1) * D]
                sh = shat[:, n * D:(n + 1) * D]
                nc.vector.tensor_tensor(vmq[:], kv["V"][:], qs, op=OP.subtract)
                nc.vector.scalar_tensor_tensor(
                    sh, in0=vmq[:], scalar=attn[:, n:n + 1], in1=qs,
                    op0=OP.mult, op1=OP.add)
                nc.vector.scalar_tensor_tensor(
                    junk2[:], in0=sh, scalar=1.0, in1=sh,
                    op0=OP.mult, op1=OP.mult, accum_out=ss[:, n:n + 1])
                yield

            rn = newton_rsqrt07(ss)
            yield

            shat_fin = p1.tile([128, NV * D], BF, tag="shat_fin")
            for n in range(NV):
                nc.vector.tensor_scalar(shat_fin[:, n * D:(n + 1) * D],
                                        shat[:, n * D:(n + 1) * D],
                                        rn[:, n:n + 1], None, op0=OP.mult)
                if n % 4 == 3:
                    yield
            for b in range(B_LOC):
                nc.gpsimd.dma_start(
                    scratch_flat[c * 32:(c + 1) * 32,
                                 b * NV * D:(b + 1) * NV * D],
                    shat_fin[b * 32:(b + 1) * 32, :])

        def spT_xbar(c):
            """A^T for chunk c: spT[p=d%128, h*2048 + t*64 + s] =
            scratch[c*32+t, s, h*128+p]."""
            spT = spx.tile([128, 2 * 32 * I_DIM], BF, tag="spT")
            for h in range(2):
                src = scratch[c * 32:(c + 1) * 32, :, h * 128:(h + 1) * 128]
                nc.sync.dma_start_transpose(
                    spT[:, h * 2048:(h + 1) * 2048],
                    src.rearrange("t s d -> (t s) d"))
            return spT

        state = {"spT": None, "s_strip": None, "s_prev": None,
                 "pend": []}

        def spT_slice(t):
            tq = t % 32
            # [128, (h=2, s=64)] view of A_t^T
            v = state["spT"][:].rearrange("p (h ts) -> p h ts", h=2)
            return v[:, :, tq * 64:(tq + 1) * 64]

        def phase2_step(t):
            tq8 = t % 8
            if state["pend"]:
                nc.sync.dma_start(*state["pend"].pop(0))
            if tq8 == 0:
                state["s_prev"] = state["s_strip"]
                state["s_strip"] = p2.tile([128, 8 * 128], BF, tag="s_strip", name="s_strip")
            s_strip = state["s_strip"]
            dst = s_strip[:, tq8 * 128:(tq8 + 1) * 128]
            if t == 0:
                nc.vector.tensor_scalar(
                    dst.rearrange("p (h s) -> p h s", h=2),
                    spT_slice(t), 1.0 / ALPHA, None, op0=OP.mult)
            else:
                if tq8 == 0:
                    prev = state["s_prev"][:, 7 * 128:8 * 128]
                else:
                    prev = s_strip[:, (tq8 - 1) * 128:tq8 * 128]
                tp = ps1.tile([128, 128], F32, tag="tp")
                for h in range(2):
                    for j in range(2):
                        nc.tensor.matmul(
                            tp[:, h * 64:(h + 1) * 64],
                            lhsT=wtT[:, j * D + h * 128:j * D + (h + 1) * 128],
                            rhs=prev[:, j * 64:(j + 1) * 64],
                            start=(j == 0), stop=(j == 1))
                th = p2.tile([128, 128], BF, tag="th")
                nc.scalar.activation(th[:], tp[:], AF.Tanh)
                nc.vector.scalar_tensor_tensor(
                    dst.rearrange("p (h s) -> p h s", h=2),
                    in0=th[:].rearrange("p (h s) -> p h s", h=2),
                    scalar=1.0 - ALPHA, in1=spT_slice(t),
                    op0=OP.mult, op1=OP.add)
            if tq8 == 7:
                # natural layout: s_nat[h*64+s, k*128 + d] = S_{t0+k}[h*128+d, s]
                s_nat = p2.tile([128, 8 * 128], BF, tag="s_nat")
                nc.sync.dma_start_transpose(
                    s_nat[:].rearrange("p (k d) -> p k d", d=128), s_strip[:])
                t0 = t - 7
                for b in range(B_LOC):
                    for h in range(2):
                        dst3 = out_v[b, t0:t0 + 8, :,
                                     h * 128:(h + 1) * 128].rearrange(
                                         "k n d -> n k d")
                        p0 = h * 64 + b * NV
                        src3 = s_nat[p0:p0 + NV, :].rearrange(
                            "n (k d) -> n k d", d=128)
                        state["pend"].append((dst3, src3))

        def run_gen(g):
            if g is not None:
                for _ in g:
                    pass

        ob0 = phase1_load(0)
        ob1 = phase1_load(1)
        run_gen(phase1_compute(0, ob0))
        state["spT"] = spT_xbar(0)
        run_gen(phase1_compute(1, ob1))
        spT_next = spT_xbar(1)

        for c in range(n_chunks):
            gen = None
            spT_next2 = None
            for t in range(c * 32, (c + 1) * 32):
                if t % 32 == 0 and c + 2 < n_chunks:
                    obs_next = phase1_load(c + 2)
                    gen = phase1_compute(c + 2, obs_next)
                phase2_step(t)
                # interleave phase-1 issue: ~2 quanta per step
                if gen is not None:
                    for _ in range(2):
                        if next(gen, "END") == "END":
                            gen = None
                            spT_next2 = spT_xbar(c + 2)
                            break
            if gen is not None:
                run_gen(gen)
                spT_next2 = spT_xbar(c + 2)
            state["spT"] = spT_next
            spT_next = spT_next2
        while state["pend"]:
            nc.sync.dma_start(*state["pend"].pop(0))

    nc.compile()
    return nc


def _get_nc():
    if "nc" not in _CACHE:
        _CACHE["nc"] = build(T_FULL)
    return _CACHE["nc"]


def kernel(observations, W_enc, b_enc, W_key, W_value, W_query, b_query,
           W_temporal):
    from concourse.bass_utils import run_bass_kernel_spmd

    nc = _get_nc()
    common = {
        "W_enc": np.ascontiguousarray(W_enc, np.float32),
        "b_enc": np.ascontiguousarray(b_enc, np.float32).reshape(D, 1),
        "W_key": np.ascontiguousarray(W_key, np.float32),
        "W_value": np.ascontiguousarray(W_value, np.float32),
        "W_query": np.ascontiguousarray(W_query, np.float32),
        "b_query": np.ascontiguousarray(b_query, np.float32).reshape(1, NV * D),
        "W_temporal": np.ascontiguousarray(W_temporal, np.float32),
    }
    obs = np.ascontiguousarray(observations, np.float32)
    in_maps = [
        dict(common,
             observations=np.ascontiguousarray(obs[c * B_LOC:(c + 1) * B_LOC]))
        for c in range(N_CORES)
    ]
    res = run_bass_kernel_spmd(nc, in_maps, list(range(N_CORES)))
    out = np.empty((B, T_FULL, NV, D), np.float32)
    for c in range(N_CORES):
        out[c * B_LOC:(c + 1) * B_LOC] = np.asarray(
            res.results[c]["out"], dtype=np.float32)
    return out

